# revision 1
# baseline (speedup 1.0000x reference)
"""DigitCaps dynamic-routing kernel for 8 Trainium2 NeuronCores.

Sharding:
  - s_j / squash / Wc: data-parallel over batch (B=256 -> 32 per core).
  - a_ij/b_ij update: sharded over ROUTES (1152 -> 144 per core), computed on the
    FULL batch, which needs v from every core -> AllGather(v) [32,160]->[256,160],
    and the resulting b-update slices are AllGather'd back [10,144]->[80,144].
    Two cheap AllGathers replace an AllReduce and cut the a-phase DVE/PE work 8x.

Algebra (never materialize u_hat = [B,R,C,O] 189MB):
  s_j[b,co] = sum_{(r,i)} xT[(r,i),b] * (c_ij[r,c]*W4[(r,i),co])    (K=9216 matmul)
  ab[r,c]   = (1/B) sum_{i,o} W4[(r,i),co] * G[(r,i),co],
  G         = sum_{all b} x[b,(r,i)] v[b,co]     (K=256 as two K=128 matmuls)
  then a DVE o-reduction and one PE matmul vs block-ones that i-reduces AND
  transposes, DMA'd straight into the collective bounce buffer.

Per-(r,i) partition mapping: chunk k covers routes 16k..16k+15, partition
p = 8*(r-16k)+i.  72 global chunks; each core owns local chunks 0..8.
"""

import sys
import numpy as np

sys.path.insert(0, "/opt/trn_rl_repo")

import concourse.bass as bass
import concourse.bacc as bacc
import concourse.mybir as mybir
import concourse.tile as tile
from concourse import bass_utils

F32 = mybir.dt.float32
F32R = mybir.dt.float32r
ALU = mybir.AluOpType
ACTF = mybir.ActivationFunctionType
AX = mybir.AxisListType

# s_j matmuls in float32r with 256-padded moving slots: 4 cycles/row -> 1.
# Costs ~1e-4 relative error end-to-end; False = full fp32 (~3e-6).
SJ_F32R = False

B, R, C, O, I = 256, 1152, 10, 16, 8
NCORES = 8
NB = B // NCORES            # 32 batch per core
RI = R * I                  # 9216 contraction dim
CO = C * O                  # 160 output cols
NCHUNK = RI // 128          # 72 chunks of 128 partitions
NGRP = 12                   # chunk groups (6 chunks each) for W4/Wc tiles
GC = NCHUNK // NGRP         # 6 chunks per group
GW = GC * CO                # 960 f32 per group
PSB = 512                   # PSUM bank size in f32
NLC = 9                     # local chunks per core (route shard)
RL = R // NCORES            # 144 local routes
WSPLIT = (1, 3, 4, 4)       # groups per w4/xt mega-tile (first alone -> early start)
SJP = 256 if SJ_F32R else CO   # w4/wc chunk-slot pitch (f32r needs moving dim >= 256)
SJDT = F32R if SJ_F32R else F32
GW2 = GC * SJP              # padded group width

_BUILT = None


def _warm_pe(tc, pools, src):
    """Tiny dummy matmul keyed on `src` so the PE HAM never sees an idle window."""
    nc = tc.nc
    wp = pools["warm"].tile([64, 2], F32, tag="warm", name="warm")
    nc.tensor.matmul(wp[:], lhsT=src[:, :64], rhs=src[:, :2])


def _squash(tc, pools, s_ps, scale):
    """v = sq*t/((1+sq)*sqrt(sq)) with t = scale*s, sq = t*t.  Returns SBUF tile [NB, CO]."""
    nc = tc.nc
    sb = pools["sb"]
    t = sb.tile([NB, CO], F32, tag="sq_t")
    sq = sb.tile([NB, CO], F32, tag="sq_sq")
    at = sb.tile([NB, CO], F32, tag="sq_at")
    num = sb.tile([NB, CO], F32, tag="sq_num")
    den = sb.tile([NB, CO], F32, tag="sq_den")
    rv = sb.tile([NB, CO], F32, tag="sq_rv")
    v = sb.tile([NB, CO], F32, tag="sq_v", bufs=2)
    s_ap = s_ps[:, :CO]
    # v = (t*|t|) / (1+t^2) with t = scale*s  (== sq*t/((1+sq)*sqrt(sq)) in reals)
    nc.scalar.activation(sq[:], s_ap, ACTF.Square, scale=scale)  # t^2
    nc.scalar.sqrt(at[:], sq[:])                              # |t|
    nc.scalar.mul(t[:], s_ap, scale)                          # t
    _warm_pe(tc, pools, sq)
    nc.vector.tensor_scalar_add(den[:], sq[:], 1.0)           # 1+t^2
    nc.vector.reciprocal(rv[:], den[:])
    nc.vector.tensor_mul(num[:], t[:], at[:])                 # t*|t|
    _warm_pe(tc, pools, num)
    nc.vector.tensor_mul(v[:], num[:], rv[:])
    _warm_pe(tc, pools, v)
    return v


def _sj_matmuls(tc, pools, xtv, rhsv):
    """s[b, co] = sum over all 72 chunks: xt_chunk^T @ rhs_chunk.  Returns PSUM tile.

    Under SJ_F32R the rhs slots are 256 wide (cols 160.. are zero pad) and the
    matmuls run in float32r; cols >= 160 of the PSUM tile are ignored."""
    nc = tc.nc
    width = rhsv(0).shape[-1]
    s_ps = pools["pbig"].tile([NB, SJP], F32, tag="gbig")
    for k in range(NCHUNK):
        nc.tensor.matmul(
            s_ps[:, :width],
            lhsT=xtv(k),
            rhs=rhsv(k),
            start=(k == 0),
            stop=(k == NCHUNK - 1),
        )
    return s_ps


def _ab_round(tc, pools, xr_t, w4s, v, bones, tag):
    """Full-batch route-sharded b-update round.

    AllGather v -> G over all 256 batches for our 9 local chunks -> P/o/i
    reductions -> AllGather the [10,144] slice -> return b-update [10,1152]."""
    nc = tc.nc
    pbig, psm, sb, dram = pools["pbig"], pools["psm"], pools["sb"], pools["dram"]

    ccv_in = dram.tile([NB, CO], F32, tag="ccvin", bufs=2, name=f"ccvin{tag}")
    ccv_out = dram.tile(
        [B, CO], F32, tag="ccvout", addr_space="Shared", bufs=2, name=f"ccvout{tag}"
    )
    nc.sync.dma_start(ccv_in[:], v[:])
    nc.gpsimd.collective_compute(
        "AllGather",
        ALU.bypass,
        replica_groups=[list(range(NCORES))],
        ins=[ccv_in[:].opt()],
        outs=[ccv_out[:].opt()],
    )
    vh = []
    for h in range(2):
        vt = sb.tile([128, CO], F32, tag=f"vh{h}", bufs=2, name=f"vh{h}")
        nc.sync.dma_start(vt[:], ccv_out[128 * h : 128 * h + 128, :])
        vh.append(vt)

    # G for 9 local chunks (full batch, K=256 as two accumulating K=128 matmuls)
    pr = sb.tile([128, NLC * C], F32, tag="pr", bufs=2, name=f"pr{tag}")
    for grp, (c0, nch) in enumerate(((0, 6), (6, 3))):
        g_ps = pbig.tile([128, 2 * PSB], F32, tag="gbig")
        for j in range(nch):
            off = PSB * (j // 3) + CO * (j % 3)
            for h in range(2):
                nc.tensor.matmul(
                    g_ps[:, off : off + CO],
                    lhsT=xr_t[h][:, 128 * (c0 + j) : 128 * (c0 + j) + 128],
                    rhs=vh[h][:],
                    start=(h == 0),
                    stop=(h == 1),
                )
        # P = (G/B) .* W4slice
        p_t = sb.tile([128, nch * CO], F32, tag="p", bufs=2)
        na = nch // 3
        g_view = g_ps[:].rearrange("p (a x) -> p a x", a=2)[:, :na, : 3 * CO].rearrange(
            "p a (s e) -> p a s e", s=3
        )
        w_view = w4s[:, CO * c0 : CO * (c0 + nch)].rearrange(
            "p (a s e) -> p a s e", a=na, s=3
        )
        p_view = p_t[:].rearrange("p (a s e) -> p a s e", a=na, s=3)
        nc.vector.scalar_tensor_tensor(
            p_view, g_view, 1.0 / B, w_view, ALU.mult, ALU.mult
        )
        # o-reduce into pr, c-major: free idx 9c + (c0+j)
        pr_view = (
            pr[:]
            .rearrange("p (c a) -> p c a", c=C)[:, :, c0 : c0 + nch]
            .transpose([0, 2, 1])
        )
        nc.vector.tensor_reduce(
            pr_view,
            p_t[:].rearrange("p (a o) -> p a o", o=O),
            axis=AX.X,
            op=ALU.add,
        )
    # i-reduce + transpose all 9 chunks in one matmul: out[9c+a, n] = ab[16a+n, c]
    q_ps = psm.tile([NLC * C, 16], F32, tag="sm", name="q_ps")
    nc.tensor.matmul(q_ps[:], lhsT=pr[:], rhs=bones[:])
    q_sb = sb.tile([NLC * C, 16], F32, tag="q_sb", bufs=2, name="q_sb")
    nc.scalar.copy(q_sb[:], q_ps[:])

    ccab_in = dram.tile([C, RL], F32, tag="ccabin", bufs=2, name=f"ccabin{tag}")
    ccab_out = dram.tile(
        [NCORES * C, RL], F32, tag="ccabout", addr_space="Shared", bufs=2,
        name=f"ccabout{tag}",
    )
    dst = ccab_in[:].rearrange("c (a n) -> c a n", a=NLC)
    nc.sync.dma_start(dst, q_sb[:])
    nc.gpsimd.collective_compute(
        "AllGather",
        ALU.bypass,
        replica_groups=[list(range(NCORES))],
        ins=[ccab_in[:].opt()],
        outs=[ccab_out[:].opt()],
    )
    # gather back as [10, 1152]: b[c, 144*rho + ri] = out[10*rho + c, ri]
    ar = sb.tile([C, R], F32, tag="ar", bufs=2, name=f"ar{tag}")
    src = ccab_out[:].rearrange("(rho c) ri -> c rho ri", c=C)
    nc.sync.dma_start(ar[:].rearrange("c (rho ri) -> c rho ri", rho=NCORES), src)
    return ar


def _softmax_cb_wc(tc, pools, bT, w4v, ident, selrepb, wc_t):
    """c = softmax(b) over routes; build Wc tiles = W4 .* broadcast(c)."""
    nc = tc.nc
    sb, psm = pools["sb"], pools["psm"]
    NRB = R // 128  # 9 route-blocks
    e = sb.tile([C, R], F32, tag="smx_e")
    ssum = sb.tile([C, 1], F32, tag="smx_s")
    sinv = sb.tile([C, 1], F32, tag="smx_si")
    nc.scalar.activation(e[:], bT[:], ACTF.Exp, accum_out=ssum[:])
    nc.vector.reciprocal(sinv[:], ssum[:])
    nc.vector.tensor_scalar_mul(e[:], e[:], sinv[:])  # e becomes c^T [10, 1152]
    # transpose c to route-major [128, 9*10]: col rb*10+c holds c[128rb+q, c]
    cr_all = sb.tile([128, NRB * C], F32, tag="cr_all", name="cr_all")
    for rb in range(NRB):
        tp = psm.tile([128, C], F32, tag="sm", name="tp")
        nc.tensor.transpose(tp[:], e[:, 128 * rb : 128 * rb + 128], ident[:C, :C])
        nc.scalar.copy(cr_all[:, C * rb : C * rb + C], tp[:])
    # replicate over i: for each s in 0..8, one matmul gives cb for chunks k=8rb+s:
    # out[p,(rb,c)] = cr_all[16s + p//8, (rb,c)].  Stored so col k*10+c = chunk k.
    cb_all = sb.tile([128, NCHUNK * C], F32, tag="cb_all", name="cb_all")
    cb_v = cb_all[:].rearrange("p (rb s c) -> p rb s c", s=8, c=C)
    for s in range(8):
        cb_ps = psm.tile([128, NRB * C], F32, tag="sm", name="cb_ps")
        nc.tensor.matmul(cb_ps[:], lhsT=selrepb[:, 128 * s : 128 * s + 128], rhs=cr_all[:])
        nc.scalar.copy(
            cb_v[:, :, s, :], cb_ps[:].rearrange("p (rb c) -> p rb c", c=C)
        )
    # per group: broadcast over o (step-0 AP), multiply into Wc
    for g in range(NGRP):
        cb_view = (
            cb_all[:, GC * C * g : GC * C * (g + 1)]
            .rearrange("p (j c) -> p j c", c=C)
            .unsqueeze(-1)
            .broadcast_to([128, GC, C, O])
        )
        w_view = w4v(g).rearrange("p (j c o) -> p j c o", j=GC, c=C)
        wc_view = (
            wc_t[g][:]
            .rearrange("p (j x) -> p j x", x=SJP)[:, :, :CO]
            .rearrange("p j (c o) -> p j c o", c=C)
        )
        nc.vector.tensor_mul(wc_view, w_view, cb_view)


def build():
    """Build the Bass module (one program, SPMD across 8 cores)."""
    nc = bacc.Bacc("TRN2", target_bir_lowering=False, debug=False, num_devices=NCORES)

    # chunk-major host layouts: free idx = 32k+b (xt), 160k+co (w4)
    d_xt = nc.dram_tensor("xt", [128, NCHUNK * NB], SJDT, kind="ExternalInput").ap()
    d_xr = nc.dram_tensor("xr", [B, RL * I], F32, kind="ExternalInput").ap()
    d_w4 = nc.dram_tensor("w4", [128, NCHUNK * CO], F32, kind="ExternalInput").ap()
    d_w4s = nc.dram_tensor("w4s", [128, NLC * CO], F32, kind="ExternalInput").ap()
    d_id = nc.dram_tensor("ident", [128, 128], F32, kind="ExternalInput").ap()
    d_sr = nc.dram_tensor("selrep", [128, 8 * 128], F32, kind="ExternalInput").ap()
    d_bo = nc.dram_tensor("bones", [128, 16], F32, kind="ExternalInput").ap()
    d_out = nc.dram_tensor("vout", [NB, CO], F32, kind="ExternalOutput").ap()

    with tile.TileContext(nc) as tc:
        with (
            tc.tile_pool(name="const", bufs=1) as const,
            tc.tile_pool(name="w4p", bufs=1) as w4p,
            tc.tile_pool(name="xtp", bufs=1) as xtp,
            tc.tile_pool(name="wcp", bufs=1) as wcp,
            tc.tile_pool(name="sb", bufs=1) as sb,
            tc.tile_pool(name="pbig", bufs=2, space="PSUM") as pbig,
            tc.tile_pool(name="psm", bufs=2, space="PSUM") as psm,
            tc.tile_pool(name="warm", bufs=1, space="PSUM") as warm,
            tc.tile_pool(name="dram", bufs=1, space="DRAM") as dram,
        ):
            pools = {"sb": sb, "pbig": pbig, "psm": psm, "warm": warm, "dram": dram}

            # warm up the collective engine + absorb launch skew with a throwaway
            # 32B AllGather, issued FIRST (on the otherwise-idle gpsimd queue) so
            # it completes during the input-DMA stream, long before the first
            # real collective.
            cw_in = dram.tile([1, 8], F32, tag="cwin", name="cw_in")
            cw_out = dram.tile(
                [8, 8], F32, tag="cwout", addr_space="Shared", name="cw_out"
            )
            cw_sb = sb.tile([1, 8], F32, tag="cw_sb")
            nc.gpsimd.memset(cw_sb[:], 0.0)
            nc.gpsimd.dma_start(cw_in[:], cw_sb[:])
            nc.gpsimd.collective_compute(
                "AllGather",
                ALU.bypass,
                replica_groups=[list(range(NCORES))],
                ins=[cw_in[:].opt()],
                outs=[cw_out[:].opt()],
            )

            # mega-tiles: WSPLIT groups each; first holds group 0 alone so its
            # DMA lands fast and s_j starts while the rest streams.  w4 on the
            # sync HWDGE queue, xt on the scalar HWDGE queue (both fast).
            w4_mt, xt_mt = [], []
            goff = 0
            for ti, ngr in enumerate(WSPLIT):
                wt = w4p.tile([128, ngr * GW], F32, tag=f"w4m{ti}", name=f"w4m{ti}")
                nc.sync.dma_start(wt[:], d_w4[:, GW * goff : GW * (goff + ngr)])
                xt_ = xtp.tile(
                    [128, ngr * GC * NB], SJDT, tag=f"xtm{ti}", name=f"xtm{ti}"
                )
                nc.scalar.dma_start(
                    xt_[:], d_xt[:, GC * NB * goff : GC * NB * (goff + ngr)]
                )
                w4_mt.append((goff, wt))
                xt_mt.append((goff, xt_))
                goff += ngr

            def w4v(g):
                for off, wt in reversed(w4_mt):
                    if g >= off:
                        return wt[:, GW * (g - off) : GW * (g - off + 1)]

            def xtv(k):
                g, j = k // GC, k % GC
                for off, xt_ in reversed(xt_mt):
                    if g >= off:
                        return xt_[:, NB * (GC * (g - off) + j) : NB * (GC * (g - off) + j) + NB]

            wc_t = []
            for g in range(NGRP):
                w = wcp.tile([128, GW2], SJDT, tag=f"wc_{g}", name=f"wc_{g}")
                if SJ_F32R:
                    nc.vector.memset(
                        w[:].rearrange("p (j x) -> p j x", x=SJP)[:, :, CO:].bitcast(F32),
                        0.0,
                    )
                wc_t.append(w)

            # route-shard inputs (needed from the first a-phase on)
            xr_t = []
            for h in range(2):
                xh = sb.tile([128, RL * I], F32, tag=f"xr{h}", name=f"xr{h}")
                nc.scalar.dma_start(xh[:], d_xr[128 * h : 128 * h + 128, :])
                xr_t.append(xh)
            w4s = sb.tile([128, NLC * CO], F32, tag="w4s")
            nc.scalar.dma_start(w4s[:], d_w4s[:])
            ident = const.tile([128, 128], F32, tag="ident")
            bones = const.tile([128, 16], F32, tag="bones")
            selrepb = const.tile([128, 8 * 128], F32, tag="selrepb")
            nc.scalar.dma_start(bones[:], d_bo[:])
            nc.scalar.dma_start(ident[:], d_id[:])
            nc.scalar.dma_start(selrepb[:], d_sr[:])

            # ---- iter 0: c uniform -> s = (X @ W4) / R
            # iter-0 s_j runs plain fp32 against compact W4 (it is DMA-paced anyway)
            xtv0 = (lambda k: xtv(k).bitcast(F32)) if SJ_F32R else xtv
            s0 = _sj_matmuls(tc, pools, xtv0, lambda k: w4v(k // GC)[:, CO * (k % GC) : CO * (k % GC) + CO])
            v0 = _squash(tc, pools, s0, 1.0 / R)
            bT1 = _ab_round(tc, pools, xr_t, w4s, v0, bones, tag=0)  # b was 0

            # ---- iter 1
            _softmax_cb_wc(tc, pools, bT1, w4v, ident, selrepb, wc_t)
            s1 = _sj_matmuls(tc, pools, xtv, lambda k: wc_t[k // GC][:, SJP * (k % GC) : SJP * (k % GC) + SJP])
            v1 = _squash(tc, pools, s1, 1.0)
            ar1 = _ab_round(tc, pools, xr_t, w4s, v1, bones, tag=1)
            bT2 = sb.tile([C, R], F32, tag="bT2")
            nc.vector.tensor_add(bT2[:], bT1[:], ar1[:])

            # ---- iter 2 (final)
            _softmax_cb_wc(tc, pools, bT2, w4v, ident, selrepb, wc_t)
            s2 = _sj_matmuls(tc, pools, xtv, lambda k: wc_t[k // GC][:, SJP * (k % GC) : SJP * (k % GC) + SJP])
            v2 = _squash(tc, pools, s2, 1.0)
            nc.sync.dma_start(d_out[:], v2[:])

    nc.compile()
    return nc


def _host_inputs(x, W):
    """Per-core input maps with pre-arranged layouts."""
    x = np.asarray(x, dtype=np.float32)
    W = np.asarray(W, dtype=np.float32)
    # W4[(r,i), (c,o)] chunk-major in the free dim: [128, 72*160]
    wf = np.ascontiguousarray(W.transpose(0, 3, 1, 2)).reshape(RI, CO)
    wfc = wf.reshape(NCHUNK, 128, CO)
    w4h = np.ascontiguousarray(wfc.transpose(1, 0, 2)).reshape(128, NCHUNK * CO)
    ident = np.eye(128, dtype=np.float32)
    # selrep[s][q, p] = 1 iff q == 16s + p//8, packed s-major in free dim
    selrep = np.zeros((8, 128, 128), dtype=np.float32)
    ss, qq, pp = np.meshgrid(np.arange(8), np.arange(128), np.arange(128), indexing="ij")
    selrep[qq == 16 * ss + pp // 8] = 1.0
    selrepb = np.ascontiguousarray(selrep.transpose(1, 0, 2)).reshape(128, 8 * 128)
    pq, jq = np.meshgrid(np.arange(128), np.arange(16), indexing="ij")
    bones = (pq // 8 == jq).astype(np.float32)
    in_maps = []
    for c in range(NCORES):
        xc = np.ascontiguousarray(x[NB * c : NB * (c + 1)].reshape(NB, RI))
        xt = np.ascontiguousarray(xc.T)  # [9216, 32]
        xth = np.ascontiguousarray(
            xt.reshape(NCHUNK, 128, NB).transpose(1, 0, 2)
        ).reshape(128, NCHUNK * NB)
        xrh = np.ascontiguousarray(x[:, RL * c : RL * (c + 1), :]).reshape(B, RL * I)
        w4sh = np.ascontiguousarray(
            wfc[NLC * c : NLC * (c + 1)].transpose(1, 0, 2)
        ).reshape(128, NLC * CO)
        in_maps.append(
            {
                "xt": xth,
                "xr": xrh,
                "w4": w4h,
                "w4s": w4sh,
                "ident": ident,
                "selrep": selrepb,
                "bones": bones,
            }
        )
    return in_maps


def kernel(x, W, trace=False):
    global _BUILT
    if _BUILT is None:
        _BUILT = build()
    nc = _BUILT
    in_maps = _host_inputs(x, W)
    res = bass_utils.run_bass_kernel_spmd(
        nc, in_maps, core_ids=list(range(NCORES)), trace=trace
    )
    v = np.concatenate([res.results[c]["vout"] for c in range(NCORES)], axis=0)
    out = v.reshape(B, C, O, 1)
    if trace:
        kernel.last_exec_time_ns = res.exec_time_ns
        kernel.last_results = res
    return out



# revision 7
# speedup vs baseline: 1.1126x; 1.1126x over previous
"""DigitCaps dynamic-routing kernel for 8 Trainium2 NeuronCores (v2, bf16).

Sharding: data-parallel over batch (B=256 -> 32 per core), W replicated.

Key changes vs v1:
  - All big matmuls in bf16 (1 cycle/row on PE vs 4 for fp32): s_j, G, cb.
  - b-update restructured to ONE AllReduce per routing round (was two
    AllGathers): each core computes ab[r,c] for ALL routes from its LOCAL
    32 batches via K=32 matmuls G[(r,i),co] = x_loc^T @ v_loc, then
    P = (G/B) .* W4 -> o-reduce -> i-reduce (block-ones matmul) and a single
    AllReduce(add) of the [10,1152] partial b-updates.
  - squash is pure-DVE (abs via abs_max) - no scalar act-table thrash; the
    scalar engine only ever runs Exp.
  - Wc build and the P/o-reduce phases alternate between Vector and GpSimd.

Algebra (u_hat = [B,R,C,O] is never materialized):
  s_j[b,co] = sum_{(r,i)} xT[(r,i),b] * (c_ij[r,c]*W4[(r,i),co])   (K=9216)
  ab[r,c]   = (1/B) sum_{i,o} W4[(r,i),co] * G[(r,i),co],
  G         = sum_{local b} x[b,(r,i)] v[b,co]                      (K=32)

Per-(r,i) partition mapping: chunk k covers routes 16k..16k+15, partition
p = 8*(r-16k)+i.  72 chunks of 128 partitions.
"""

import sys
import numpy as np

sys.path.insert(0, "/opt/trn_rl_repo")

import concourse.bass as bass
import concourse.bacc as bacc
import concourse.mybir as mybir
import concourse.tile as tile
from concourse import bass_utils

F32 = mybir.dt.float32
BF16 = mybir.dt.bfloat16
ALU = mybir.AluOpType
ACTF = mybir.ActivationFunctionType
AX = mybir.AxisListType

B, R, C, O, I = 256, 1152, 10, 16, 8
NCORES = 8
NB = B // NCORES            # 32 batch per core
RI = R * I                  # 9216 contraction dim
CO = C * O                  # 160 output cols
NCHUNK = RI // 128          # 72 chunks of 128 partitions
NGRP = 12                   # chunk groups (6 chunks each) for W4/Wc tiles
GC = NCHUNK // NGRP         # 6 chunks per group
GW = GC * CO                # 960 elems per group
NTRI = NCHUNK // 3          # 24 triples (G/P granularity)
NBLK = NCHUNK // 9          # 8 blocks (i-reduce granularity)
WSPLIT = (1, 3, 4, 4)       # groups per w4/xt mega-tile (first alone -> early start)

_BUILT = None


def _warm_pe(tc, pools, src):
    """Tiny dummy matmul keyed on `src` so the PE never sees a long idle gap."""
    nc = tc.nc
    wp = pools["warm"].tile([64, 2], F32, tag="warm", name="warm")
    nc.tensor.matmul(wp[:], lhsT=src[:, :64], rhs=src[:, :2])


def _squash(tc, pools, s_ps, scale):
    """v = (t*|t|)/(1+t^2) with t = scale*s.  Pure DVE.  Returns SBUF [NB,CO] f32."""
    nc = tc.nc
    sb = pools["sb"]
    t = sb.tile([NB, CO], F32, tag="sq_t")
    s2 = sb.tile([NB, CO], F32, tag="sq_s2")
    at = sb.tile([NB, CO], F32, tag="sq_at")
    num = sb.tile([NB, CO], F32, tag="sq_num")
    den = sb.tile([NB, CO], F32, tag="sq_den")
    rv = sb.tile([NB, CO], F32, tag="sq_rv")
    v = sb.tile([NB, CO], F32, tag="sq_v", bufs=2)
    ng = sb.tile([NB, CO], F32, tag="sq_ng")
    nc.vector.tensor_scalar_mul(t[:], s_ps[:, :CO], scale)       # t (PSUM->SBUF)
    nc.vector.tensor_mul(s2[:], t[:], t[:])                      # t^2
    nc.vector.tensor_scalar_mul(ng[:], t[:], -1.0)               # -t
    nc.vector.tensor_max(at[:], t[:], ng[:])                     # |t|
    _warm_pe(tc, pools, s2)
    nc.vector.tensor_scalar_add(den[:], s2[:], 1.0)              # 1+t^2
    nc.vector.reciprocal(rv[:], den[:])
    nc.vector.tensor_mul(num[:], t[:], at[:])                    # t*|t|
    _warm_pe(tc, pools, num)
    nc.vector.tensor_mul(v[:], num[:], rv[:])
    _warm_pe(tc, pools, v)
    return v


def _sj_matmuls(tc, pools, xtv, rhsv):
    """s[b,co] = sum over 72 chunks xt_k^T @ rhs_k (bf16).  Returns PSUM tile."""
    nc = tc.nc
    s_ps = pools["ps"].tile([NB, CO], F32, tag="s")
    for k in range(NCHUNK):
        nc.tensor.matmul(
            s_ps[:],
            lhsT=xtv(k),
            rhs=rhsv(k),
            start=(k == 0),
            stop=(k == NCHUNK - 1),
        )
    return s_ps


def _ab_round(tc, pools, xl, w4v3, v, bones, tag):
    """Local-batch b-update round: G (K=32) -> P -> o/i-reduce -> AllReduce.

    Returns SBUF tile bdel [10, 1152] = sum over cores of (1/B)*a_ij partial."""
    nc = tc.nc
    pg, psm, sb, dram = pools["pg"], pools["psm"], pools["sb"], pools["dram"]

    vb = sb.tile([NB, CO], BF16, tag="vb", bufs=2)
    nc.scalar.copy(vb[:], v[:])

    pr = sb.tile([128, NBLK * 9 * C], F32, tag="pr", bufs=2, name=f"pr{tag}")
    ccin = dram.tile([C, R], F32, tag="ccin", bufs=2, name=f"ccin{tag}")
    ccout = dram.tile(
        [C, R], F32, tag="ccout", addr_space="Shared", bufs=2, name=f"ccout{tag}"
    )

    for tri in range(NTRI):
        g_ps = pg.tile([128, 3 * CO], F32, tag="g")
        for j in range(3):
            k = 3 * tri + j
            nc.tensor.matmul(
                g_ps[:, CO * j : CO * (j + 1)],
                lhsT=xl[:, 128 * k : 128 * (k + 1)],
                rhs=vb[:],
            )
        p_t = sb.tile([128, 3 * CO], BF16, tag="p", bufs=4)
        nc.vector.scalar_tensor_tensor(
            p_t[:], g_ps[:], 1.0 / B, w4v3(tri), ALU.mult, ALU.mult
        )
        # o-reduce: out free idx (within 90-col block) = 9c + a_local
        blk, t3 = tri // 3, tri % 3
        pr_view = (
            pr[:, 90 * blk : 90 * (blk + 1)]
            .rearrange("p (c a) -> p c a", c=C)[:, :, 3 * t3 : 3 * t3 + 3]
            .transpose([0, 2, 1])
        )
        nc.vector.tensor_reduce(
            pr_view,
            p_t[:].rearrange("p (a o) -> p a o", o=O),
            axis=AX.X,
            op=ALU.add,
        )
        if tri % 3 == 2:
            # i-reduce + transpose for this block of 9 chunks:
            # q[9c+a, n] = ab[route 16*(9blk+a)+n, c] / B
            q_ps = psm.tile([9 * C, 16], F32, tag="sm", name="q_ps")
            nc.tensor.matmul(q_ps[:], lhsT=pr[:, 90 * blk : 90 * (blk + 1)], rhs=bones[:])
            q_sb = sb.tile([9 * C, 16], F32, tag="q_sb", bufs=4, name="q_sb")
            nc.scalar.copy(q_sb[:], q_ps[:])
            dst = ccin[:, 144 * blk : 144 * (blk + 1)].rearrange(
                "c (a n) -> c a n", a=9
            )
            nc.sync.dma_start(dst, q_sb[:])

    nc.gpsimd.collective_compute(
        "AllReduce",
        ALU.add,
        replica_groups=[list(range(NCORES))],
        ins=[ccin[:].opt()],
        outs=[ccout[:].opt()],
    )
    bdel = sb.tile([C, R], F32, tag="bdel", bufs=2, name=f"bdel{tag}")
    nc.sync.dma_start(bdel[:], ccout[:])
    return bdel


def _softmax_cb_wc(tc, pools, bT, w4v, ident, selrepb, wc_t):
    """c = softmax(b) over routes; build Wc tiles = W4 .* broadcast(c) (bf16)."""
    nc = tc.nc
    sb, psm = pools["sb"], pools["psm"]
    NRB = R // 128  # 9 route-blocks
    e = sb.tile([C, R], F32, tag="smx_e")
    ssum = sb.tile([C, 1], F32, tag="smx_s")
    sinv = sb.tile([C, 1], F32, tag="smx_si")
    nc.scalar.activation(e[:], bT[:], ACTF.Exp, accum_out=ssum[:])
    nc.vector.reciprocal(sinv[:], ssum[:])
    nc.vector.tensor_scalar_mul(e[:], e[:], sinv[:])  # e becomes c^T [10, 1152]
    # transpose c to route-major bf16 [128, 9*10]: col rb*10+c holds c[128rb+q, c]
    cr_bf = sb.tile([128, NRB * C], BF16, tag="cr_bf", name="cr_bf")
    for rb in range(NRB):
        tp = psm.tile([128, C], F32, tag="sm", name="tp")
        nc.tensor.transpose(tp[:], e[:, 128 * rb : 128 * rb + 128], ident[:C, :C])
        nc.scalar.copy(cr_bf[:, C * rb : C * rb + C], tp[:])
    # replicate over i: for each s in 0..7, one matmul gives cb for chunks k=8rb+s:
    # out[p,(rb,c)] = cr_bf[16s + p//8, (rb,c)].  Stored so col k*10+c = chunk k.
    cb_all = sb.tile([128, NCHUNK * C], BF16, tag="cb_all", name="cb_all")
    cb_v = cb_all[:].rearrange("p (rb s c) -> p rb s c", s=8, c=C)
    for s in range(8):
        cb_ps = psm.tile([128, NRB * C], F32, tag="sm", name="cb_ps")
        nc.tensor.matmul(cb_ps[:], lhsT=selrepb[:, 128 * s : 128 * s + 128], rhs=cr_bf[:])
        nc.scalar.copy(
            cb_v[:, :, s, :], cb_ps[:].rearrange("p (rb c) -> p rb c", c=C)
        )
    # per group: broadcast over o (step-0 AP), multiply into Wc (bf16), split
    # across Vector and GpSimd
    for g in range(NGRP):
        cb_view = (
            cb_all[:, GC * C * g : GC * C * (g + 1)]
            .rearrange("p (j c) -> p j c", c=C)
            .unsqueeze(-1)
            .broadcast_to([128, GC, C, O])
        )
        w_view = w4v(g).rearrange("p (j c o) -> p j c o", j=GC, c=C)
        wc_view = wc_t[g][:].rearrange("p (j c o) -> p j c o", j=GC, c=C)
        eng = nc.vector if g % 2 == 0 else nc.gpsimd
        eng.tensor_mul(wc_view, w_view, cb_view)


def build():
    """Build the Bass module (one program, SPMD across 8 cores)."""
    nc = bacc.Bacc("TRN2", target_bir_lowering=False, debug=False, num_devices=NCORES)

    # chunk-major host layouts: free idx = 32k+b (xt), 160k+co (w4)
    d_xt = nc.dram_tensor("xt", [128, NCHUNK * NB], BF16, kind="ExternalInput").ap()
    d_xl = nc.dram_tensor("xl", [NB, RI], BF16, kind="ExternalInput").ap()
    d_w4 = nc.dram_tensor("w4", [128, NCHUNK * CO], BF16, kind="ExternalInput").ap()
    d_id = nc.dram_tensor("ident", [128, 128], F32, kind="ExternalInput").ap()
    d_sr = nc.dram_tensor("selrep", [128, 8 * 128], BF16, kind="ExternalInput").ap()
    d_bo = nc.dram_tensor("bones", [128, 16], F32, kind="ExternalInput").ap()
    d_out = nc.dram_tensor("vout", [NB, CO], F32, kind="ExternalOutput").ap()

    with tile.TileContext(nc) as tc:
        with (
            tc.tile_pool(name="const", bufs=1) as const,
            tc.tile_pool(name="w4p", bufs=1) as w4p,
            tc.tile_pool(name="xtp", bufs=1) as xtp,
            tc.tile_pool(name="wcp", bufs=1) as wcp,
            tc.tile_pool(name="sb", bufs=1) as sb,
            tc.tile_pool(name="pg", bufs=3, space="PSUM") as pg,
            tc.tile_pool(name="ps", bufs=2, space="PSUM") as ps,
            tc.tile_pool(name="psm", bufs=2, space="PSUM") as psm,
            tc.tile_pool(name="warm", bufs=1, space="PSUM") as warm,
            tc.tile_pool(name="dram", bufs=1, space="DRAM") as dram,
        ):
            pools = {
                "sb": sb, "pg": pg, "ps": ps, "psm": psm, "warm": warm, "dram": dram,
            }

            # warm up the collective stream + absorb launch skew with two
            # throwaway 32B AllGathers issued FIRST on the gpsimd queue.
            cw_sb = sb.tile([1, 8], F32, tag="cw_sb")
            nc.gpsimd.memset(cw_sb[:], 0.0)
            for wi in range(2):
                cw_in = dram.tile([1, 8], F32, tag="cwin", name=f"cw_in{wi}")
                cw_out = dram.tile(
                    [8, 8], F32, tag="cwout", addr_space="Shared", name=f"cw_out{wi}"
                )
                nc.gpsimd.dma_start(cw_in[:], cw_sb[:])
                nc.gpsimd.collective_compute(
                    "AllGather",
                    ALU.bypass,
                    replica_groups=[list(range(NCORES))],
                    ins=[cw_in[:].opt()],
                    outs=[cw_out[:].opt()],
                )

            # mega-tiles: WSPLIT groups each; first holds group 0 alone so its
            # DMA lands fast and s_j starts while the rest streams.  w4 on the
            # sync HWDGE queue, xt on the scalar HWDGE queue.
            w4_mt, xt_mt = [], []
            goff = 0
            for ti, ngr in enumerate(WSPLIT):
                wt = w4p.tile([128, ngr * GW], BF16, tag=f"w4m{ti}", name=f"w4m{ti}")
                nc.sync.dma_start(wt[:], d_w4[:, GW * goff : GW * (goff + ngr)])
                xt_ = xtp.tile([128, ngr * GC * NB], BF16, tag=f"xtm{ti}", name=f"xtm{ti}")
                nc.scalar.dma_start(
                    xt_[:], d_xt[:, GC * NB * goff : GC * NB * (goff + ngr)]
                )
                w4_mt.append((goff, wt))
                xt_mt.append((goff, xt_))
                goff += ngr

            def w4v(g):
                for off, wt in reversed(w4_mt):
                    if g >= off:
                        return wt[:, GW * (g - off) : GW * (g - off + 1)]

            def w4v3(tri):
                g = tri // 2
                h = tri % 2
                return w4v(g)[:, 480 * h : 480 * (h + 1)]

            def xtv(k):
                g, j = k // GC, k % GC
                for off, xt_ in reversed(xt_mt):
                    if g >= off:
                        return xt_[:, NB * (GC * (g - off) + j) : NB * (GC * (g - off) + j) + NB]

            # local-batch x for G (a-phase), and constants
            xl = sb.tile([NB, RI], BF16, tag="xl", name="xl")
            nc.scalar.dma_start(xl[:], d_xl[:])
            ident = const.tile([128, 128], F32, tag="ident")
            bones = const.tile([128, 16], F32, tag="bones")
            selrepb = const.tile([128, 8 * 128], BF16, tag="selrepb")
            nc.scalar.dma_start(bones[:], d_bo[:])
            nc.scalar.dma_start(ident[:], d_id[:])
            nc.scalar.dma_start(selrepb[:], d_sr[:])

            wc_t = [
                wcp.tile([128, GW], BF16, tag=f"wc_{g}", name=f"wc_{g}")
                for g in range(NGRP)
            ]

            # ---- iter 0: c uniform -> s = (X @ W4), squash scale 1/R
            s0 = _sj_matmuls(
                tc, pools, xtv,
                lambda k: w4v(k // GC)[:, CO * (k % GC) : CO * (k % GC) + CO],
            )
            v0 = _squash(tc, pools, s0, 1.0 / R)
            bT1 = _ab_round(tc, pools, xl, w4v3, v0, bones, tag=0)  # b was 0

            # ---- iter 1
            _softmax_cb_wc(tc, pools, bT1, w4v, ident, selrepb, wc_t)
            s1 = _sj_matmuls(
                tc, pools, xtv,
                lambda k: wc_t[k // GC][:, CO * (k % GC) : CO * (k % GC) + CO],
            )
            v1 = _squash(tc, pools, s1, 1.0)
            ar1 = _ab_round(tc, pools, xl, w4v3, v1, bones, tag=1)
            bT2 = sb.tile([C, R], F32, tag="bT2")
            nc.vector.tensor_add(bT2[:], bT1[:], ar1[:])

            # ---- iter 2 (final)
            _softmax_cb_wc(tc, pools, bT2, w4v, ident, selrepb, wc_t)
            s2 = _sj_matmuls(
                tc, pools, xtv,
                lambda k: wc_t[k // GC][:, CO * (k % GC) : CO * (k % GC) + CO],
            )
            v2 = _squash(tc, pools, s2, 1.0)
            nc.sync.dma_start(d_out[:], v2[:])

    nc.compile()
    return nc


def _host_inputs(x, W):
    """Per-core input maps with pre-arranged layouts (bf16 for the big ones)."""
    import ml_dtypes

    bf16 = ml_dtypes.bfloat16
    x = np.asarray(x, dtype=np.float32)
    W = np.asarray(W, dtype=np.float32)
    # W4[(r,i), (c,o)] chunk-major in the free dim: [128, 72*160]
    wf = np.ascontiguousarray(W.transpose(0, 3, 1, 2)).reshape(RI, CO)
    wfc = wf.reshape(NCHUNK, 128, CO)
    w4h = np.ascontiguousarray(wfc.transpose(1, 0, 2)).reshape(128, NCHUNK * CO)
    w4h = w4h.astype(bf16)
    ident = np.eye(128, dtype=np.float32)
    # selrep[s][q, p] = 1 iff q == 16s + p//8, packed s-major in free dim
    selrep = np.zeros((8, 128, 128), dtype=np.float32)
    ss, qq, pp = np.meshgrid(np.arange(8), np.arange(128), np.arange(128), indexing="ij")
    selrep[qq == 16 * ss + pp // 8] = 1.0
    selrepb = np.ascontiguousarray(selrep.transpose(1, 0, 2)).reshape(128, 8 * 128)
    selrepb = selrepb.astype(bf16)
    pq, jq = np.meshgrid(np.arange(128), np.arange(16), indexing="ij")
    bones = (pq // 8 == jq).astype(np.float32)
    in_maps = []
    for c in range(NCORES):
        xc = np.ascontiguousarray(x[NB * c : NB * (c + 1)].reshape(NB, RI))
        xt = np.ascontiguousarray(xc.T)  # [9216, 32]
        xth = np.ascontiguousarray(
            xt.reshape(NCHUNK, 128, NB).transpose(1, 0, 2)
        ).reshape(128, NCHUNK * NB)
        in_maps.append(
            {
                "xt": xth.astype(bf16),
                "xl": xc.astype(bf16),
                "w4": w4h,
                "ident": ident,
                "selrep": selrepb,
                "bones": bones,
            }
        )
    return in_maps


def kernel(x, W, trace=False):
    global _BUILT
    if _BUILT is None:
        _BUILT = build()
    nc = _BUILT
    in_maps = _host_inputs(x, W)
    res = bass_utils.run_bass_kernel_spmd(
        nc, in_maps, core_ids=list(range(NCORES)), trace=trace
    )
    v = np.concatenate([res.results[c]["vout"] for c in range(NCORES)], axis=0)
    out = v.reshape(B, C, O, 1)
    if trace:
        kernel.last_exec_time_ns = res.exec_time_ns
        kernel.last_results = res
    return out


# revision 15
# speedup vs baseline: 1.1692x; 1.0508x over previous
"""DigitCaps dynamic-routing kernel for 8 Trainium2 NeuronCores (v2, bf16).

Sharding: data-parallel over batch (B=256 -> 32 per core), W replicated.

Key changes vs v1:
  - All big matmuls in bf16 (1 cycle/row on PE vs 4 for fp32): s_j, G, cb.
  - b-update restructured to ONE AllReduce per routing round (was two
    AllGathers): each core computes ab[r,c] for ALL routes from its LOCAL
    32 batches via K=32 matmuls G[(r,i),co] = x_loc^T @ v_loc, then
    P = (G/B) .* W4 -> o-reduce -> i-reduce (block-ones matmul) and a single
    AllReduce(add) of the [10,1152] partial b-updates.
  - squash is pure-DVE (abs via abs_max) - no scalar act-table thrash; the
    scalar engine only ever runs Exp.
  - Wc build and the P/o-reduce phases alternate between Vector and GpSimd.

Algebra (u_hat = [B,R,C,O] is never materialized):
  s_j[b,co] = sum_{(r,i)} xT[(r,i),b] * (c_ij[r,c]*W4[(r,i),co])   (K=9216)
  ab[r,c]   = (1/B) sum_{i,o} W4[(r,i),co] * G[(r,i),co],
  G         = sum_{local b} x[b,(r,i)] v[b,co]                      (K=32)

Per-(r,i) partition mapping: chunk k covers routes 16k..16k+15, partition
p = 8*(r-16k)+i.  72 chunks of 128 partitions.
"""

import sys
import numpy as np

sys.path.insert(0, "/opt/trn_rl_repo")

import concourse.bass as bass
import concourse.bacc as bacc
import concourse.mybir as mybir
import concourse.tile as tile
from concourse import bass_utils

F32 = mybir.dt.float32
BF16 = mybir.dt.bfloat16
ALU = mybir.AluOpType
ACTF = mybir.ActivationFunctionType
AX = mybir.AxisListType

B, R, C, O, I = 256, 1152, 10, 16, 8
NCORES = 8
NB = B // NCORES            # 32 batch per core
RI = R * I                  # 9216 contraction dim
CO = C * O                  # 160 output cols
NCHUNK = RI // 128          # 72 chunks of 128 partitions
NGRP = 12                   # chunk groups (6 chunks each) for W4/Wc tiles
GC = NCHUNK // NGRP         # 6 chunks per group
GW = GC * CO                # 960 elems per group
NTRI = NCHUNK // 3          # 24 triples (G/P granularity)
NBLK = NCHUNK // 9          # 8 blocks (i-reduce granularity)
WSPLIT = (1, 3, 4, 4)       # groups per w4/xt mega-tile (first alone -> early start)

_BUILT = None


def _warm_pe(tc, pools, src):
    """Tiny dummy matmul keyed on `src` so the PE never sees a long idle gap."""
    nc = tc.nc
    wp = pools["warm"].tile([64, 2], F32, tag="warm", name="warm")
    nc.tensor.matmul(wp[:], lhsT=src[:, :64], rhs=src[:, :2])


def _squash(tc, pools, s_ps, scale):
    """v = (t*|t|)/(1+t^2) with t = scale*s.  Pure DVE.  Returns SBUF [NB,CO] f32."""
    nc = tc.nc
    sb = pools["sb"]
    t = sb.tile([NB, CO], F32, tag="sq_t")
    s2 = sb.tile([NB, CO], F32, tag="sq_s2")
    at = sb.tile([NB, CO], F32, tag="sq_at")
    num = sb.tile([NB, CO], F32, tag="sq_num")
    den = sb.tile([NB, CO], F32, tag="sq_den")
    rv = sb.tile([NB, CO], F32, tag="sq_rv")
    v = sb.tile([NB, CO], F32, tag="sq_v", bufs=2)
    ng = sb.tile([NB, CO], F32, tag="sq_ng")
    nc.vector.tensor_scalar_mul(t[:], s_ps[:, :CO], scale)       # t (PSUM->SBUF)
    nc.vector.tensor_mul(s2[:], t[:], t[:])                      # t^2
    nc.vector.tensor_scalar_mul(ng[:], t[:], -1.0)               # -t
    nc.vector.tensor_max(at[:], t[:], ng[:])                     # |t|
    _warm_pe(tc, pools, s2)
    nc.vector.tensor_scalar_add(den[:], s2[:], 1.0)              # 1+t^2
    nc.vector.reciprocal(rv[:], den[:])
    nc.vector.tensor_mul(num[:], t[:], at[:])                    # t*|t|
    _warm_pe(tc, pools, num)
    nc.vector.tensor_mul(v[:], num[:], rv[:])
    _warm_pe(tc, pools, v)
    return v


def _sj_matmuls(tc, pools, xtv, rhsv):
    """s[b,co] = sum over 72 chunks xt_k^T @ rhs_k (bf16).  Returns PSUM tile."""
    nc = tc.nc
    s_ps = pools["ps"].tile([NB, CO], F32, tag="s")
    for k in range(NCHUNK):
        nc.tensor.matmul(
            s_ps[:],
            lhsT=xtv(k),
            rhs=rhsv(k),
            start=(k == 0),
            stop=(k == NCHUNK - 1),
        )
    return s_ps


def _ab_round(tc, pools, xl, w4v3, v, bones, tag):
    """Local-batch b-update round: G (K=32) -> P -> o/i-reduce -> AllReduce.

    Returns SBUF tile bdel [10, 1152] = sum over cores of (1/B)*a_ij partial."""
    nc = tc.nc
    pg, psm, sb, dram = pools["pg"], pools["psm"], pools["sb"], pools["dram"]

    vb = sb.tile([NB, CO], BF16, tag="vb", bufs=2)
    nc.scalar.copy(vb[:], v[:])

    pr = sb.tile([128, NBLK * 9 * C], F32, tag="pr", bufs=2, name=f"pr{tag}")
    ccin = dram.tile([C, R], F32, tag="ccin", bufs=2, name=f"ccin{tag}")
    ccout = dram.tile(
        [C, R], F32, tag="ccout", addr_space="Shared", bufs=2, name=f"ccout{tag}"
    )

    for tri in range(NTRI):
        g_ps = pg.tile([128, 3 * CO], F32, tag="g")
        for j in range(3):
            k = 3 * tri + j
            nc.tensor.matmul(
                g_ps[:, CO * j : CO * (j + 1)],
                lhsT=xl[:, 128 * k : 128 * (k + 1)],
                rhs=vb[:],
            )
        p_t = sb.tile([128, 3 * CO], BF16, tag="p", bufs=4)
        nc.vector.scalar_tensor_tensor(
            p_t[:], g_ps[:], 1.0 / B, w4v3(tri), ALU.mult, ALU.mult
        )
        # o-reduce: out free idx (within 90-col block) = 9c + a_local
        blk, t3 = tri // 3, tri % 3
        pr_view = (
            pr[:, 90 * blk : 90 * (blk + 1)]
            .rearrange("p (c a) -> p c a", c=C)[:, :, 3 * t3 : 3 * t3 + 3]
            .transpose([0, 2, 1])
        )
        nc.vector.tensor_reduce(
            pr_view,
            p_t[:].rearrange("p (a o) -> p a o", o=O),
            axis=AX.X,
            op=ALU.add,
        )
        if tri % 3 == 2:
            # i-reduce + transpose for this block of 9 chunks:
            # q[9c+a, n] = ab[route 16*(9blk+a)+n, c] / B
            q_ps = psm.tile([9 * C, 16], F32, tag="sm", name="q_ps")
            nc.tensor.matmul(q_ps[:], lhsT=pr[:, 90 * blk : 90 * (blk + 1)], rhs=bones[:])
            q_sb = sb.tile([9 * C, 16], F32, tag="q_sb", bufs=4, name="q_sb")
            nc.scalar.copy(q_sb[:], q_ps[:])
            dst = ccin[:, 144 * blk : 144 * (blk + 1)].rearrange(
                "c (a n) -> c a n", a=9
            )
            nc.sync.dma_start(dst, q_sb[:])

    nc.gpsimd.collective_compute(
        "AllReduce",
        ALU.add,
        replica_groups=[list(range(NCORES))],
        ins=[ccin[:].opt()],
        outs=[ccout[:].opt()],
    )
    bdel = sb.tile([C, R], F32, tag="bdel", bufs=2, name=f"bdel{tag}")
    nc.sync.dma_start(bdel[:], ccout[:])
    return bdel


def _ab_round_sharded(tc, pools, xr_t, w4s, v, bones, tag):
    """Route-sharded b-update round (for use once the CC stream is warm).

    AllGather v (bf16) -> G over all 256 batches for our 9 local chunks ->
    P/o/i reductions -> AllGather the [10,144] slice -> return [10,1152]."""
    nc = tc.nc
    pg, psm, sb, dram = pools["pg"], pools["psm"], pools["sb"], pools["dram"]

    vb = sb.tile([NB, CO], BF16, tag="vb", bufs=2)
    nc.scalar.copy(vb[:], v[:])
    ccv_in = dram.tile([NB, CO], BF16, tag="ccvin", bufs=2, name=f"ccvin{tag}")
    ccv_out = dram.tile(
        [B, CO], BF16, tag="ccvout", addr_space="Shared", bufs=2, name=f"ccvout{tag}"
    )
    nc.sync.dma_start(ccv_in[:], vb[:])
    nc.gpsimd.collective_compute(
        "AllGather",
        ALU.bypass,
        replica_groups=[list(range(NCORES))],
        ins=[ccv_in[:].opt()],
        outs=[ccv_out[:].opt()],
    )
    vh = []
    for h in range(2):
        vt = sb.tile([128, CO], BF16, tag=f"vh{h}", bufs=2, name=f"vh{h}")
        nc.sync.dma_start(vt[:], ccv_out[128 * h : 128 * h + 128, :])
        vh.append(vt)

    # G for 9 local chunks (full batch, K=256 as two accumulating K=128 matmuls)
    pr = sb.tile([128, 9 * C], F32, tag="prs", bufs=2, name=f"prs{tag}")
    for tri in range(3):
        g_ps = pg.tile([128, 3 * CO], F32, tag="g")
        for j in range(3):
            k = 3 * tri + j
            for h in range(2):
                nc.tensor.matmul(
                    g_ps[:, CO * j : CO * (j + 1)],
                    lhsT=xr_t[h][:, 128 * k : 128 * (k + 1)],
                    rhs=vh[h][:],
                    start=(h == 0),
                    stop=(h == 1),
                )
        p_t = sb.tile([128, 3 * CO], BF16, tag="p", bufs=4)
        nc.vector.scalar_tensor_tensor(
            p_t[:], g_ps[:], 1.0 / B, w4s[:, 480 * tri : 480 * (tri + 1)],
            ALU.mult, ALU.mult,
        )
        pr_view = (
            pr[:]
            .rearrange("p (c a) -> p c a", c=C)[:, :, 3 * tri : 3 * tri + 3]
            .transpose([0, 2, 1])
        )
        nc.vector.tensor_reduce(
            pr_view,
            p_t[:].rearrange("p (a o) -> p a o", o=O),
            axis=AX.X,
            op=ALU.add,
        )
    q_ps = psm.tile([9 * C, 16], F32, tag="sm", name="q_ps")
    nc.tensor.matmul(q_ps[:], lhsT=pr[:], rhs=bones[:])
    q_sb = sb.tile([9 * C, 16], F32, tag="q_sb", bufs=4, name="q_sbs")
    nc.scalar.copy(q_sb[:], q_ps[:])

    ccab_in = dram.tile([C, R // NCORES], F32, tag="ccabin", bufs=2, name=f"ccabin{tag}")
    ccab_out = dram.tile(
        [NCORES * C, R // NCORES], F32, tag="ccabout", addr_space="Shared", bufs=2,
        name=f"ccabout{tag}",
    )
    dst = ccab_in[:].rearrange("c (a n) -> c a n", a=9)
    nc.sync.dma_start(dst, q_sb[:])
    nc.gpsimd.collective_compute(
        "AllGather",
        ALU.bypass,
        replica_groups=[list(range(NCORES))],
        ins=[ccab_in[:].opt()],
        outs=[ccab_out[:].opt()],
    )
    # gather back as [10, 1152]: b[c, 144*rho + ri] = out[10*rho + c, ri]
    ar = sb.tile([C, R], F32, tag="ar", bufs=2, name=f"ar{tag}")
    src = ccab_out[:].rearrange("(rho c) ri -> c rho ri", c=C)
    nc.sync.dma_start(ar[:].rearrange("c (rho ri) -> c rho ri", rho=NCORES), src)
    return ar


def _softmax_cb_wc(tc, pools, bT, w4v, ident, selrepb, wc_t):
    """c = softmax(b) over routes; build Wc tiles = W4 .* broadcast(c) (bf16)."""
    nc = tc.nc
    sb, psm = pools["sb"], pools["psm"]
    NRB = R // 128  # 9 route-blocks
    e = sb.tile([C, R], F32, tag="smx_e")
    ssum = sb.tile([C, 1], F32, tag="smx_s")
    sinv = sb.tile([C, 1], F32, tag="smx_si")
    nc.scalar.activation(e[:], bT[:], ACTF.Exp, accum_out=ssum[:])
    nc.vector.reciprocal(sinv[:], ssum[:])
    nc.vector.tensor_scalar_mul(e[:], e[:], sinv[:])  # e becomes c^T [10, 1152]
    # transpose c to route-major bf16 [128, 9*10]: col rb*10+c holds c[128rb+q, c]
    cr_bf = sb.tile([128, NRB * C], BF16, tag="cr_bf", name="cr_bf")
    for rb in range(NRB):
        tp = psm.tile([128, C], F32, tag="sm", name="tp")
        nc.tensor.transpose(tp[:], e[:, 128 * rb : 128 * rb + 128], ident[:C, :C])
        nc.scalar.copy(cr_bf[:, C * rb : C * rb + C], tp[:])
    # replicate over i: for each s in 0..7, one matmul gives cb for chunks k=8rb+s:
    # out[p,(rb,c)] = cr_bf[16s + p//8, (rb,c)].  Stored so col k*10+c = chunk k.
    cb_all = sb.tile([128, NCHUNK * C], BF16, tag="cb_all", name="cb_all")
    cb_v = cb_all[:].rearrange("p (rb s c) -> p rb s c", s=8, c=C)
    for s in range(8):
        cb_ps = psm.tile([128, NRB * C], F32, tag="sm", name="cb_ps")
        nc.tensor.matmul(cb_ps[:], lhsT=selrepb[:, 128 * s : 128 * s + 128], rhs=cr_bf[:])
        nc.scalar.copy(
            cb_v[:, :, s, :], cb_ps[:].rearrange("p (rb c) -> p rb c", c=C)
        )
    # per group: broadcast over o (step-0 AP), multiply into Wc (bf16), split
    # across Vector and GpSimd
    for g in range(NGRP):
        cb_view = (
            cb_all[:, GC * C * g : GC * C * (g + 1)]
            .rearrange("p (j c) -> p j c", c=C)
            .unsqueeze(-1)
            .broadcast_to([128, GC, C, O])
        )
        w_view = w4v(g).rearrange("p (j c o) -> p j c o", j=GC, c=C)
        wc_view = wc_t[g][:].rearrange("p (j c o) -> p j c o", j=GC, c=C)
        eng = nc.gpsimd if g >= 9 else nc.vector
        eng.tensor_mul(wc_view, w_view, cb_view)


def build():
    """Build the Bass module (one program, SPMD across 8 cores)."""
    nc = bacc.Bacc("TRN2", target_bir_lowering=False, debug=False, num_devices=NCORES)

    # chunk-major host layouts: free idx = 32k+b (xt), 160k+co (w4)
    d_xt = nc.dram_tensor("xt", [128, NCHUNK * NB], BF16, kind="ExternalInput").ap()
    d_xl = nc.dram_tensor("xl", [NB, RI], BF16, kind="ExternalInput").ap()
    d_xr = nc.dram_tensor("xr", [B, R // NCORES * I], BF16, kind="ExternalInput").ap()
    d_w4s = nc.dram_tensor("w4s", [128, 9 * CO], BF16, kind="ExternalInput").ap()
    d_w4 = nc.dram_tensor("w4", [128, NCHUNK * CO], BF16, kind="ExternalInput").ap()
    d_id = nc.dram_tensor("ident", [128, 128], F32, kind="ExternalInput").ap()
    d_sr = nc.dram_tensor("selrep", [128, 8 * 128], BF16, kind="ExternalInput").ap()
    d_bo = nc.dram_tensor("bones", [128, 16], F32, kind="ExternalInput").ap()
    d_out = nc.dram_tensor("vout", [NB, CO], F32, kind="ExternalOutput").ap()

    with tile.TileContext(nc) as tc:
        with (
            tc.tile_pool(name="const", bufs=1) as const,
            tc.tile_pool(name="w4p", bufs=1) as w4p,
            tc.tile_pool(name="xtp", bufs=1) as xtp,
            tc.tile_pool(name="wcp", bufs=1) as wcp,
            tc.tile_pool(name="sb", bufs=1) as sb,
            tc.tile_pool(name="pg", bufs=3, space="PSUM") as pg,
            tc.tile_pool(name="ps", bufs=2, space="PSUM") as ps,
            tc.tile_pool(name="psm", bufs=2, space="PSUM") as psm,
            tc.tile_pool(name="warm", bufs=1, space="PSUM") as warm,
            tc.tile_pool(name="dram", bufs=1, space="DRAM") as dram,
        ):
            pools = {
                "sb": sb, "pg": pg, "ps": ps, "psm": psm, "warm": warm, "dram": dram,
            }

            # mega-tiles: WSPLIT groups each; first holds group 0 alone so its
            # DMA lands fast and s_j starts while the rest streams.  w4 on the
            # sync HWDGE queue, xt on the scalar HWDGE queue.
            w4_mt, xt_mt = [], []
            goff = 0
            for ti, ngr in enumerate(WSPLIT):
                wt = w4p.tile([128, ngr * GW], BF16, tag=f"w4m{ti}", name=f"w4m{ti}")
                nc.sync.dma_start(wt[:], d_w4[:, GW * goff : GW * (goff + ngr)])
                xt_ = xtp.tile([128, ngr * GC * NB], BF16, tag=f"xtm{ti}", name=f"xtm{ti}")
                nc.scalar.dma_start(
                    xt_[:], d_xt[:, GC * NB * goff : GC * NB * (goff + ngr)]
                )
                w4_mt.append((goff, wt))
                xt_mt.append((goff, xt_))
                goff += ngr

            def w4v(g):
                for off, wt in reversed(w4_mt):
                    if g >= off:
                        return wt[:, GW * (g - off) : GW * (g - off + 1)]

            def w4v3(tri):
                g = tri // 2
                h = tri % 2
                return w4v(g)[:, 480 * h : 480 * (h + 1)]

            def xtv(k):
                g, j = k // GC, k % GC
                for off, xt_ in reversed(xt_mt):
                    if g >= off:
                        return xt_[:, NB * (GC * (g - off) + j) : NB * (GC * (g - off) + j) + NB]

            # local-batch x for G (a-phase), route-shard x, and constants
            xl = sb.tile([NB, RI], BF16, tag="xl", name="xl")
            nc.scalar.dma_start(xl[:], d_xl[:])
            xr_t = []
            for h in range(2):
                xh = sb.tile([128, R // NCORES * I], BF16, tag=f"xr{h}", name=f"xr{h}")
                nc.scalar.dma_start(xh[:], d_xr[128 * h : 128 * h + 128, :])
                xr_t.append(xh)
            w4s = sb.tile([128, 9 * CO], BF16, tag="w4s")
            nc.scalar.dma_start(w4s[:], d_w4s[:])
            ident = const.tile([128, 128], F32, tag="ident")
            bones = const.tile([128, 16], F32, tag="bones")
            selrepb = const.tile([128, 8 * 128], BF16, tag="selrepb")
            nc.scalar.dma_start(bones[:], d_bo[:])
            nc.scalar.dma_start(ident[:], d_id[:])
            nc.scalar.dma_start(selrepb[:], d_sr[:])

            wc_t = [
                wcp.tile([128, GW], BF16, tag=f"wc_{g}", name=f"wc_{g}")
                for g in range(NGRP)
            ]

            # ---- iter 0: c uniform -> s = (X @ W4), squash scale 1/R
            s0 = _sj_matmuls(
                tc, pools, xtv,
                lambda k: w4v(k // GC)[:, CO * (k % GC) : CO * (k % GC) + CO],
            )
            v0 = _squash(tc, pools, s0, 1.0 / R)
            bT1 = _ab_round(tc, pools, xl, w4v3, v0, bones, tag=0)  # b was 0

            # ---- iter 1
            _softmax_cb_wc(tc, pools, bT1, w4v, ident, selrepb, wc_t)
            s1 = _sj_matmuls(
                tc, pools, xtv,
                lambda k: wc_t[k // GC][:, CO * (k % GC) : CO * (k % GC) + CO],
            )
            v1 = _squash(tc, pools, s1, 1.0)
            ar1 = _ab_round_sharded(tc, pools, xr_t, w4s, v1, bones, tag=1)
            bT2 = sb.tile([C, R], F32, tag="bT2")
            nc.vector.tensor_add(bT2[:], bT1[:], ar1[:])

            # ---- iter 2 (final)
            _softmax_cb_wc(tc, pools, bT2, w4v, ident, selrepb, wc_t)
            s2 = _sj_matmuls(
                tc, pools, xtv,
                lambda k: wc_t[k // GC][:, CO * (k % GC) : CO * (k % GC) + CO],
            )
            v2 = _squash(tc, pools, s2, 1.0)
            nc.sync.dma_start(d_out[:], v2[:])

    nc.compile()
    return nc


def _host_inputs(x, W):
    """Per-core input maps with pre-arranged layouts (bf16 for the big ones)."""
    import ml_dtypes

    bf16 = ml_dtypes.bfloat16
    x = np.asarray(x, dtype=np.float32)
    W = np.asarray(W, dtype=np.float32)
    # W4[(r,i), (c,o)] chunk-major in the free dim: [128, 72*160]
    wf = np.ascontiguousarray(W.transpose(0, 3, 1, 2)).reshape(RI, CO)
    wfc = wf.reshape(NCHUNK, 128, CO)
    w4h = np.ascontiguousarray(wfc.transpose(1, 0, 2)).reshape(128, NCHUNK * CO)
    w4h = w4h.astype(bf16)
    ident = np.eye(128, dtype=np.float32)
    # selrep[s][q, p] = 1 iff q == 16s + p//8, packed s-major in free dim
    selrep = np.zeros((8, 128, 128), dtype=np.float32)
    ss, qq, pp = np.meshgrid(np.arange(8), np.arange(128), np.arange(128), indexing="ij")
    selrep[qq == 16 * ss + pp // 8] = 1.0
    selrepb = np.ascontiguousarray(selrep.transpose(1, 0, 2)).reshape(128, 8 * 128)
    selrepb = selrepb.astype(bf16)
    pq, jq = np.meshgrid(np.arange(128), np.arange(16), indexing="ij")
    bones = (pq // 8 == jq).astype(np.float32)
    in_maps = []
    for c in range(NCORES):
        xc = np.ascontiguousarray(x[NB * c : NB * (c + 1)].reshape(NB, RI))
        xt = np.ascontiguousarray(xc.T)  # [9216, 32]
        xth = np.ascontiguousarray(
            xt.reshape(NCHUNK, 128, NB).transpose(1, 0, 2)
        ).reshape(128, NCHUNK * NB)
        RL = R // NCORES
        xrh = np.ascontiguousarray(x[:, RL * c : RL * (c + 1), :]).reshape(B, RL * I)
        w4sh = np.ascontiguousarray(
            wfc[9 * c : 9 * (c + 1)].transpose(1, 0, 2)
        ).reshape(128, 9 * CO)
        in_maps.append(
            {
                "xt": xth.astype(bf16),
                "xl": xc.astype(bf16),
                "xr": xrh.astype(bf16),
                "w4s": w4sh.astype(bf16),
                "w4": w4h,
                "ident": ident,
                "selrep": selrepb,
                "bones": bones,
            }
        )
    return in_maps


def kernel(x, W, trace=False):
    global _BUILT
    if _BUILT is None:
        _BUILT = build()
    nc = _BUILT
    in_maps = _host_inputs(x, W)
    res = bass_utils.run_bass_kernel_spmd(
        nc, in_maps, core_ids=list(range(NCORES)), trace=trace
    )
    v = np.concatenate([res.results[c]["vout"] for c in range(NCORES)], axis=0)
    out = v.reshape(B, C, O, 1)
    if trace:
        kernel.last_exec_time_ns = res.exec_time_ns
        kernel.last_results = res
    return out


# revision 20
# speedup vs baseline: 1.4727x; 1.2596x over previous
"""DigitCaps dynamic-routing kernel for 8 Trainium2 NeuronCores (v4).

Sharding: 2D data-parallel over (batch x caps): 4 batch-groups x 2 cap-halves.
Core g handles batches 64*(g%4).. and caps 5*(g//4)..  W replicated per
cap-half.  This halves every per-core compute phase vs plain batch sharding,
keeping PE bursts inside the HAM full-speed window (k=8/8 lasts only ~3.5us
per burst before the activity manager throttles to k=4/8).

All big matmuls in bf16 (1 cycle/row).  b-update: each core computes
ab[r,c_local] for ALL routes from its LOCAL 64 batches via K=64 matmuls
G[(r,i),col] = x_loc^T @ v_loc, then P = (G/B) .* W4 -> o-reduce ->
i-reduce (block-ones matmul) -> ONE AllReduce(add) of [5,1152] per 4-core
cap-group per routing round.

squash is pure-DVE; the scalar engine only ever runs Exp (one table load).

Per-(r,i) partition mapping: chunk k covers routes 16k..16k+15, partition
p = 8*(r-16k)+i.  72 chunks of 128 partitions.
"""

import sys
import numpy as np

sys.path.insert(0, "/opt/trn_rl_repo")

import concourse.bass as bass
import concourse.bacc as bacc
import concourse.mybir as mybir
import concourse.tile as tile
from concourse import bass_utils

F32 = mybir.dt.float32
BF16 = mybir.dt.bfloat16
ALU = mybir.AluOpType
ACTF = mybir.ActivationFunctionType
AX = mybir.AxisListType

B, R, C, O, I = 256, 1152, 10, 16, 8
NCORES = 8
NBG = 4                     # batch groups
NCH = 2                     # cap halves
NBL = B // NBG              # 64 batches per core
CL = C // NCH               # 5 caps per core
COL = CL * O                # 80 output cols per core
RI = R * I                  # 9216 contraction dim
NCHUNK = RI // 128          # 72 chunks of 128 partitions
NGRP = 12                   # chunk groups (6 chunks each) for W4/Wc tiles
GC = NCHUNK // NGRP         # 6 chunks per group
GW = GC * COL               # 480 elems per group
NTRI = NCHUNK // 3          # 24 triples (G/P granularity)
NBLK = NCHUNK // 9          # 8 blocks (i-reduce granularity)
WSPLIT = (1, 3, 4, 4)       # groups per w4/xt mega-tile (first alone -> early start)
CC_GROUPS = [[0, 1, 2, 3], [4, 5, 6, 7]]   # cap-half AllReduce groups

_BUILT = None


def _warm_pe(tc, pools, src):
    """Tiny dummy matmul keyed on `src` so the PE never sees a long idle gap."""
    nc = tc.nc
    wp = pools["warm"].tile([64, 2], F32, tag="warm", name="warm")
    nc.tensor.matmul(wp[:], lhsT=src[:, :64], rhs=src[:, :2])


def _squash(tc, pools, s_ps, scale):
    """v = (t*|t|)/(1+t^2) with t = scale*s.  Pure DVE.  Returns SBUF [NBL,COL]."""
    nc = tc.nc
    sb = pools["sb"]
    t = sb.tile([NBL, COL], F32, tag="sq_t")
    s2 = sb.tile([NBL, COL], F32, tag="sq_s2")
    at = sb.tile([NBL, COL], F32, tag="sq_at")
    ng = sb.tile([NBL, COL], F32, tag="sq_ng")
    num = sb.tile([NBL, COL], F32, tag="sq_num")
    den = sb.tile([NBL, COL], F32, tag="sq_den")
    rv = sb.tile([NBL, COL], F32, tag="sq_rv")
    v = sb.tile([NBL, COL], F32, tag="sq_v", bufs=2)
    nc.vector.tensor_scalar_mul(t[:], s_ps[:, :COL], scale)      # t (PSUM->SBUF)
    nc.vector.tensor_mul(s2[:], t[:], t[:])                      # t^2
    nc.vector.tensor_scalar_mul(ng[:], t[:], -1.0)               # -t
    nc.vector.tensor_max(at[:], t[:], ng[:])                     # |t|
    _warm_pe(tc, pools, s2)
    nc.vector.tensor_scalar_add(den[:], s2[:], 1.0)              # 1+t^2
    nc.vector.reciprocal(rv[:], den[:])
    nc.vector.tensor_mul(num[:], t[:], at[:])                    # t*|t|
    _warm_pe(tc, pools, num)
    nc.vector.tensor_mul(v[:], num[:], rv[:])
    _warm_pe(tc, pools, v)
    return v


def _sj_matmuls(tc, pools, xtv, rhsv):
    """s[b,col] = sum over 72 chunks xt_k^T @ rhs_k (bf16).  Returns PSUM tile."""
    nc = tc.nc
    s_ps = pools["ps"].tile([NBL, COL], F32, tag="s")
    for k in range(NCHUNK):
        nc.tensor.matmul(
            s_ps[:],
            lhsT=xtv(k),
            rhs=rhsv(k),
            start=(k == 0),
            stop=(k == NCHUNK - 1),
        )
    return s_ps


def _ab_round(tc, pools, xl, w4v3, v, bones, tag):
    """Local-batch b-update round: G (K=64) -> P -> o/i-reduce -> AllReduce.

    Returns SBUF tile bdel [5, 1152] = sum over the cap-group of the
    (1/B)*a_ij partials."""
    nc = tc.nc
    pg, psm, sb, dram = pools["pg"], pools["psm"], pools["sb"], pools["dram"]

    vb = sb.tile([NBL, COL], BF16, tag="vb", bufs=2)
    nc.scalar.copy(vb[:], v[:])

    pr = sb.tile([128, NBLK * 9 * CL], F32, tag="pr", bufs=2, name=f"pr{tag}")
    ccin = dram.tile([CL, R], F32, tag="ccin", bufs=2, name=f"ccin{tag}")
    ccout = dram.tile([CL, R], F32, tag="ccout", bufs=2, name=f"ccout{tag}")

    for tri in range(NTRI):
        g_ps = pg.tile([128, 3 * COL], F32, tag="g")
        for j in range(3):
            k = 3 * tri + j
            nc.tensor.matmul(
                g_ps[:, COL * j : COL * (j + 1)],
                lhsT=xl[:, 128 * k : 128 * (k + 1)],
                rhs=vb[:],
            )
        # PSUM->SBUF extraction on the (otherwise idle) scalar engine, so the
        # vector ops below are pure-SBUF bf16 and eligible for 2x packing.
        gb = sb.tile([128, 3 * COL], BF16, tag="gb", bufs=4)
        nc.scalar.copy(gb[:], g_ps[:])
        p_t = sb.tile([128, 3 * COL], BF16, tag="p", bufs=4)
        nc.vector.scalar_tensor_tensor(
            p_t[:], gb[:], 1.0 / B, w4v3(tri), ALU.mult, ALU.mult
        )
        # o-reduce: out free idx (within 45-col block) = 9c + a_local
        blk, t3 = tri // 3, tri % 3
        pr_view = (
            pr[:, 45 * blk : 45 * (blk + 1)]
            .rearrange("p (c a) -> p c a", c=CL)[:, :, 3 * t3 : 3 * t3 + 3]
            .transpose([0, 2, 1])
        )
        nc.vector.tensor_reduce(
            pr_view,
            p_t[:].rearrange("p (a o) -> p a o", o=O),
            axis=AX.X,
            op=ALU.add,
        )
        if tri % 3 == 2:
            # i-reduce + transpose for this block of 9 chunks:
            # q[9c+a, n] = ab[route 16*(9blk+a)+n, cap c] / B
            q_ps = psm.tile([9 * CL, 16], F32, tag="sm", name="q_ps")
            nc.tensor.matmul(q_ps[:], lhsT=pr[:, 45 * blk : 45 * (blk + 1)], rhs=bones[:])
            q_sb = sb.tile([9 * CL, 16], F32, tag="q_sb", bufs=4, name="q_sb")
            nc.scalar.copy(q_sb[:], q_ps[:])
            dst = ccin[:, 144 * blk : 144 * (blk + 1)].rearrange(
                "c (a n) -> c a n", a=9
            )
            nc.sync.dma_start(dst, q_sb[:])

    nc.gpsimd.collective_compute(
        "AllReduce",
        ALU.add,
        replica_groups=CC_GROUPS,
        ins=[ccin[:].opt()],
        outs=[ccout[:].opt()],
    )
    bdel = sb.tile([CL, R], F32, tag="bdel", bufs=2, name=f"bdel{tag}")
    nc.sync.dma_start(bdel[:], ccout[:])
    return bdel


def _softmax_cb_wc(tc, pools, bT, w4v, ident, selrepb, wc_t):
    """c = softmax(b) over routes; build Wc tiles = W4 .* broadcast(c) (bf16)."""
    nc = tc.nc
    sb, psm = pools["sb"], pools["psm"]
    NRB = R // 128  # 9 route-blocks
    e = sb.tile([CL, R], F32, tag="smx_e")
    ssum = sb.tile([CL, 1], F32, tag="smx_s")
    sinv = sb.tile([CL, 1], F32, tag="smx_si")
    nc.scalar.activation(e[:], bT[:], ACTF.Exp, accum_out=ssum[:])
    nc.vector.reciprocal(sinv[:], ssum[:])
    nc.vector.tensor_scalar_mul(e[:], e[:], sinv[:])  # e becomes c^T [5, 1152]
    # transpose c to route-major bf16 [128, 9*5]: col rb*5+c holds c[128rb+q, c]
    cr_bf = sb.tile([128, NRB * CL], BF16, tag="cr_bf", name="cr_bf")
    for rb in range(NRB):
        tp = psm.tile([128, CL], F32, tag="sm", name="tp")
        nc.tensor.transpose(tp[:], e[:, 128 * rb : 128 * rb + 128], ident[:CL, :CL])
        nc.scalar.copy(cr_bf[:, CL * rb : CL * rb + CL], tp[:])
    # replicate over i: for each s in 0..7, one matmul gives cb for chunks k=8rb+s:
    # out[p,(rb,c)] = cr_bf[16s + p//8, (rb,c)].  Stored so col k*5+c = chunk k.
    cb_all = sb.tile([128, NCHUNK * CL], BF16, tag="cb_all", name="cb_all")
    cb_v = cb_all[:].rearrange("p (rb s c) -> p rb s c", s=8, c=CL)
    for s in range(8):
        cb_ps = psm.tile([128, NRB * CL], F32, tag="sm", name="cb_ps")
        nc.tensor.matmul(cb_ps[:], lhsT=selrepb[:, 128 * s : 128 * s + 128], rhs=cr_bf[:])
        nc.vector.tensor_copy(
            cb_v[:, :, s, :], cb_ps[:].rearrange("p (rb c) -> p rb c", c=CL)
        )
    # per group: broadcast over o (step-0 AP), multiply into Wc (bf16); most on
    # Vector, tail groups on GpSimd (slow per-op but runs in parallel)
    for g in range(NGRP):
        cb_view = (
            cb_all[:, GC * CL * g : GC * CL * (g + 1)]
            .rearrange("p (j c) -> p j c", c=CL)
            .unsqueeze(-1)
            .broadcast_to([128, GC, CL, O])
        )
        w_view = w4v(g).rearrange("p (j c o) -> p j c o", j=GC, c=CL)
        wc_view = wc_t[g][:].rearrange("p (j c o) -> p j c o", j=GC, c=CL)
        eng = nc.gpsimd if g >= 9 else nc.vector
        eng.tensor_mul(wc_view, w_view, cb_view)


def build():
    """Build the Bass module (one program, SPMD across 8 cores)."""
    nc = bacc.Bacc("TRN2", target_bir_lowering=False, debug=False, num_devices=NCORES)

    # chunk-major host layouts: free idx = 64k+b (xt), 80k+col (w4)
    d_xt = nc.dram_tensor("xt", [128, NCHUNK * NBL], BF16, kind="ExternalInput").ap()
    d_xl = nc.dram_tensor("xl", [NBL, RI], BF16, kind="ExternalInput").ap()
    d_w4 = nc.dram_tensor("w4", [128, NCHUNK * COL], BF16, kind="ExternalInput").ap()
    d_id = nc.dram_tensor("ident", [128, 128], F32, kind="ExternalInput").ap()
    d_sr = nc.dram_tensor("selrep", [128, 8 * 128], BF16, kind="ExternalInput").ap()
    d_bo = nc.dram_tensor("bones", [128, 16], F32, kind="ExternalInput").ap()
    d_out = nc.dram_tensor("vout", [NBL, COL], F32, kind="ExternalOutput").ap()

    with tile.TileContext(nc) as tc:
        with (
            tc.tile_pool(name="const", bufs=1) as const,
            tc.tile_pool(name="w4p", bufs=1) as w4p,
            tc.tile_pool(name="xtp", bufs=1) as xtp,
            tc.tile_pool(name="wcp", bufs=1) as wcp,
            tc.tile_pool(name="sb", bufs=1) as sb,
            tc.tile_pool(name="pg", bufs=3, space="PSUM") as pg,
            tc.tile_pool(name="ps", bufs=2, space="PSUM") as ps,
            tc.tile_pool(name="psm", bufs=2, space="PSUM") as psm,
            tc.tile_pool(name="warm", bufs=1, space="PSUM") as warm,
            tc.tile_pool(name="dram", bufs=1, space="DRAM") as dram,
        ):
            pools = {
                "sb": sb, "pg": pg, "ps": ps, "psm": psm, "warm": warm, "dram": dram,
            }

            # warm up the collective stream with a throwaway 32B AllReduce on
            # the same replica groups, issued FIRST so it absorbs the CC
            # first-op setup cost while the init barrier drains.
            cw_sb = sb.tile([1, 8], F32, tag="cw_sb")
            nc.gpsimd.memset(cw_sb[:], 0.0)
            cw_in = dram.tile([1, 8], F32, tag="cwin", name="cw_in")
            cw_out = dram.tile([1, 8], F32, tag="cwout", name="cw_out")
            nc.gpsimd.dma_start(cw_in[:], cw_sb[:])
            nc.gpsimd.collective_compute(
                "AllReduce",
                ALU.add,
                replica_groups=CC_GROUPS,
                ins=[cw_in[:].opt()],
                outs=[cw_out[:].opt()],
            )

            # mega-tiles: WSPLIT groups each; first holds group 0 alone so its
            # DMA lands fast and s_j starts while the rest streams.  w4 on the
            # sync HWDGE queue, xt on the scalar HWDGE queue.
            w4_mt, xt_mt = [], []
            goff = 0
            for ti, ngr in enumerate(WSPLIT):
                wt = w4p.tile([128, ngr * GW], BF16, tag=f"w4m{ti}", name=f"w4m{ti}")
                nc.sync.dma_start(wt[:], d_w4[:, GW * goff : GW * (goff + ngr)])
                xt_ = xtp.tile([128, ngr * GC * NBL], BF16, tag=f"xtm{ti}", name=f"xtm{ti}")
                nc.scalar.dma_start(
                    xt_[:], d_xt[:, GC * NBL * goff : GC * NBL * (goff + ngr)]
                )
                w4_mt.append((goff, wt))
                xt_mt.append((goff, xt_))
                goff += ngr

            def w4v(g):
                for off, wt in reversed(w4_mt):
                    if g >= off:
                        return wt[:, GW * (g - off) : GW * (g - off + 1)]

            def w4v3(tri):
                g = tri // 2
                h = tri % 2
                return w4v(g)[:, 240 * h : 240 * (h + 1)]

            def xtv(k):
                g, j = k // GC, k % GC
                for off, xt_ in reversed(xt_mt):
                    if g >= off:
                        return xt_[:, NBL * (GC * (g - off) + j) : NBL * (GC * (g - off) + j) + NBL]

            # local-batch x for G (a-phase), and constants
            xl = sb.tile([NBL, RI], BF16, tag="xl", name="xl")
            nc.scalar.dma_start(xl[:], d_xl[:])
            ident = const.tile([128, 128], F32, tag="ident")
            bones = const.tile([128, 16], F32, tag="bones")
            selrepb = const.tile([128, 8 * 128], BF16, tag="selrepb")
            nc.scalar.dma_start(bones[:], d_bo[:])
            nc.scalar.dma_start(ident[:], d_id[:])
            nc.scalar.dma_start(selrepb[:], d_sr[:])

            wc_t = [
                wcp.tile([128, GW], BF16, tag=f"wc_{g}", name=f"wc_{g}")
                for g in range(NGRP)
            ]

            # ---- iter 0: c uniform -> s = (X @ W4), squash scale 1/R
            s0 = _sj_matmuls(
                tc, pools, xtv,
                lambda k: w4v(k // GC)[:, COL * (k % GC) : COL * (k % GC) + COL],
            )
            v0 = _squash(tc, pools, s0, 1.0 / R)
            bT1 = _ab_round(tc, pools, xl, w4v3, v0, bones, tag=0)  # b was 0

            # ---- iter 1
            _softmax_cb_wc(tc, pools, bT1, w4v, ident, selrepb, wc_t)
            s1 = _sj_matmuls(
                tc, pools, xtv,
                lambda k: wc_t[k // GC][:, COL * (k % GC) : COL * (k % GC) + COL],
            )
            v1 = _squash(tc, pools, s1, 1.0)
            ar1 = _ab_round(tc, pools, xl, w4v3, v1, bones, tag=1)
            bT2 = sb.tile([CL, R], F32, tag="bT2")
            nc.vector.tensor_add(bT2[:], bT1[:], ar1[:])

            # ---- iter 2 (final)
            _softmax_cb_wc(tc, pools, bT2, w4v, ident, selrepb, wc_t)
            s2 = _sj_matmuls(
                tc, pools, xtv,
                lambda k: wc_t[k // GC][:, COL * (k % GC) : COL * (k % GC) + COL],
            )
            v2 = _squash(tc, pools, s2, 1.0)
            nc.sync.dma_start(d_out[:], v2[:])

    nc.compile()
    return nc


def _host_inputs(x, W):
    """Per-core input maps with pre-arranged layouts (bf16 for the big ones)."""
    import ml_dtypes

    bf16 = ml_dtypes.bfloat16
    x = np.asarray(x, dtype=np.float32)
    W = np.asarray(W, dtype=np.float32)
    # W4[(r,i), (c,o)] -> per cap-half [RI, 80], chunk-major [128, 72*80]
    wf = np.ascontiguousarray(W.transpose(0, 3, 1, 2)).reshape(RI, C, O)
    w4h = []
    for ch in range(NCH):
        wfl = np.ascontiguousarray(wf[:, CL * ch : CL * (ch + 1), :]).reshape(RI, COL)
        wfc = wfl.reshape(NCHUNK, 128, COL)
        w4h.append(
            np.ascontiguousarray(wfc.transpose(1, 0, 2))
            .reshape(128, NCHUNK * COL)
            .astype(bf16)
        )
    ident = np.eye(128, dtype=np.float32)
    # selrep[s][q, p] = 1 iff q == 16s + p//8, packed s-major in free dim
    selrep = np.zeros((8, 128, 128), dtype=np.float32)
    ss, qq, pp = np.meshgrid(np.arange(8), np.arange(128), np.arange(128), indexing="ij")
    selrep[qq == 16 * ss + pp // 8] = 1.0
    selrepb = np.ascontiguousarray(selrep.transpose(1, 0, 2)).reshape(128, 8 * 128)
    selrepb = selrepb.astype(bf16)
    pq, jq = np.meshgrid(np.arange(128), np.arange(16), indexing="ij")
    bones = (pq // 8 == jq).astype(np.float32)
    in_maps = []
    for g in range(NCORES):
        bg, ch = g % NBG, g // NBG
        xc = np.ascontiguousarray(x[NBL * bg : NBL * (bg + 1)].reshape(NBL, RI))
        xt = np.ascontiguousarray(xc.T)  # [9216, 64]
        xth = np.ascontiguousarray(
            xt.reshape(NCHUNK, 128, NBL).transpose(1, 0, 2)
        ).reshape(128, NCHUNK * NBL)
        in_maps.append(
            {
                "xt": xth.astype(bf16),
                "xl": xc.astype(bf16),
                "w4": w4h[ch],
                "ident": ident,
                "selrep": selrepb,
                "bones": bones,
            }
        )
    return in_maps


def kernel(x, W, trace=False):
    global _BUILT
    if _BUILT is None:
        _BUILT = build()
    nc = _BUILT
    in_maps = _host_inputs(x, W)
    res = bass_utils.run_bass_kernel_spmd(
        nc, in_maps, core_ids=list(range(NCORES)), trace=trace
    )
    out = np.zeros((B, C, O), dtype=np.float32)
    for g in range(NCORES):
        bg, ch = g % NBG, g // NBG
        v = res.results[g]["vout"].reshape(NBL, CL, O)
        out[NBL * bg : NBL * (bg + 1), CL * ch : CL * (ch + 1), :] = v
    if trace:
        kernel.last_exec_time_ns = res.exec_time_ns
        kernel.last_results = res
    return out[..., None]
